# revision 4
# baseline (speedup 1.0000x reference)
"""DGCNN edge-conv graph-feature module on Trainium2 (Bass/Tile) — v2.

Per-core (one batch): F-space KNN (k=20) over N=4096 points (C=64), gather
neighbor features, edge-MLP (128->128->64->64, relu), max-pool over k.

v2 changes vs baseline:
  - dist matmul in fp16 (4x PE) with hi/lo split of the -|x|^2 row
  - L1 matmuls eliminated: u_j = W1e x_j precomputed per point, staged to
    DRAM, gathered per pair by an HBM-source dma_gather (descriptor-gen
    only on GPSIMD instead of the Q7-heavy SBUF-source transpose gather)
  - L1 = relu(u_gather + v_i) via DVE broadcast-add + relu
  - L2/L3 as 512-col fp16 matmuls; PSUM evacuation (relu+bias) on ACT
  - pool on DVE fp16 tree; output transposes in a tail phase
"""

import os
import sys

for _p in ("/opt/trn_rl_repo", "/root/.axon_site/_ro/trn_rl_repo"):
    if os.path.isdir(_p) and _p not in sys.path:
        sys.path.insert(0, _p)

import numpy as np

import concourse.bass as bass
import concourse.mybir as mybir
from concourse import bacc
from concourse.bass_utils import run_bass_kernel_spmd
from concourse.masks import make_identity
from concourse.tile import TileContext

f32 = mybir.dt.float32
f16 = mybir.dt.float16
i16 = mybir.dt.int16
u16 = mybir.dt.uint16

B, N, C, K = 8, 4096, 64, 20
C1, C2, C3 = 128, 64, 64
NT = N // 128              # point tiles per core
NBLK = N // 512            # candidate blocks per tile
NCAND = NBLK * 8           # merge candidates per row
PAIRS = 128 * K            # pairs per point tile (2560)
GROUP = int(os.environ.get("KM_GROUP", "16"))  # tiles per phase group
NEG = -1e30
CA = C + 2                 # augmented contraction (64 + ones + ones)
GATHER = os.environ.get("KM_GATHER", "sbufdma")  # "ap" | "sbufdma"
SINGLE_PACKET = os.environ.get("KM_SP", "0") == "1"
VEXP_DMA = os.environ.get("KM_VEXP", "0") == "1"


def build_nc(nt=NT):
    nc = bacc.Bacc(None, target_bir_lowering=False)

    pts = nc.declare_dram_parameter("points", [N, C], f32, isOutput=False)
    w1 = nc.declare_dram_parameter("W1", [C1, 2 * C], f32, isOutput=False)
    b1 = nc.declare_dram_parameter("b1", [C1], f32, isOutput=False)
    w2 = nc.declare_dram_parameter("W2", [C2, C1], f32, isOutput=False)
    b2 = nc.declare_dram_parameter("b2", [C2], f32, isOutput=False)
    w3 = nc.declare_dram_parameter("W3", [C3, C2], f32, isOutput=False)
    b3 = nc.declare_dram_parameter("b3", [C3], f32, isOutput=False)
    out = nc.declare_dram_parameter("out", [N, C3], f32, isOutput=True)

    chain = []  # GPSIMD extended-ISA ops, chained to batch ucode libraries

    with TileContext(nc) as tc:
        with tc.tile_pool(name="const", bufs=1) as cp:
            ident = cp.tile([128, 128], f32)
            make_identity(nc, ident)
            ident16 = cp.tile([128, 128], f16)
            nc.vector.tensor_copy(ident16, ident)

            # ---- load points as [p, T, c]
            x_sb = cp.tile([128, NT, C], f32)
            nc.sync.dma_start(out=x_sb, in_=bass.AP(
                tensor=pts.ap().tensor, offset=0,
                ap=[[C, 128], [128 * C, NT], [1, C]]))

            # ---- weights + biases
            w1_sb = cp.tile([C1, 2 * C], f32)
            nc.sync.dma_start(out=w1_sb, in_=w1[:, :])
            w2_sb = cp.tile([C2, C1], f32)
            nc.sync.dma_start(out=w2_sb, in_=w2[:, :])
            w3_sb = cp.tile([C3, C2], f32)
            nc.sync.dma_start(out=w3_sb, in_=w3[:, :])
            b1_row = cp.tile([1, C1], f32)
            nc.sync.dma_start(out=b1_row, in_=b1.ap().rearrange("(a c) -> a c", a=1))
            b2_col = cp.tile([C2, 1], f32)
            nc.sync.dma_start(out=b2_col, in_=b2.ap().rearrange("(c a) -> c a", a=1))
            b3_col = cp.tile([C3, 1], f32)
            nc.sync.dma_start(out=b3_col, in_=b3.ap().rearrange("(c a) -> c a", a=1))

            # slot -> 512*block offset for globalizing block-local idx
            offs = cp.tile([128, NCAND], u16)
            nc.gpsimd.iota(offs, pattern=[[512, NBLK], [0, 8]], base=0,
                           channel_multiplier=0)

            # ---- xTaug16 [CA, N]: rows 0:64 xT, 64 ones, 65 ones
            # ---- xT2aug16 [CA, N]: rows 0:64 2*xT, 64 hi(-|x|^2), 65 lo
            xTaug16 = cp.tile([CA, N], f16)
            xT2aug16 = cp.tile([CA, N], f16)
            nc.vector.memset(xTaug16[C:CA, :], 1.0)

            # v^T [ch, T, pt]; dtype matches the gathered-u dtype for the add
            vT_all = cp.tile([128, NT, C1], f32 if GATHER == "ap" else f16)
            if GATHER == "ap":
                # u^T [ch, pt] f32 for the per-partition free-dim ap_gather
                uT_sb = cp.tile([128, N], f32)
            else:
                # u [pt, T, ch] f16 for the SBUF-source transpose dma_gather
                u_stage = cp.tile([128, NT, C1], f16)

            with tc.tile_pool(name="setup", bufs=1) as sp, \
                 tc.tile_pool(name="setup_ps", bufs=2, space="PSUM") as sps:
                # transposed weights (fp16)
                w1t16 = cp.tile([2 * C, C1], f16)
                p = sps.tile([128, 128], f32, tag="tp")
                nc.tensor.transpose(p, w1_sb, ident)
                nc.vector.tensor_copy(w1t16, p)

                w2t16 = cp.tile([C1, C2], f16)
                p = sps.tile([128, 128], f32, tag="tp")
                nc.tensor.transpose(p[:, 0:C2], w2_sb, ident[0:C2, 0:C2])
                nc.vector.tensor_copy(w2t16, p[0:C1, 0:C2])

                w3t16 = cp.tile([C2, C3], f16)
                p = sps.tile([128, 128], f32, tag="tp")
                nc.tensor.transpose(p[0:C2, 0:C3], w3_sb, ident[0:C2, 0:C2])
                nc.vector.tensor_copy(w3t16, p[0:C2, 0:C3])

                # wv16 [CA, C1]: rows 0:64 (W1c-W1e)^T, row 64 b1, row 65 0
                wv16 = cp.tile([CA, C1], f16)
                nc.vector.memset(wv16, 0.0)
                delta = sp.tile([C1, C], f32, tag="delta")
                nc.vector.tensor_tensor(
                    out=delta, in0=w1_sb[:, C:2 * C], in1=w1_sb[:, 0:C],
                    op=mybir.AluOpType.subtract)
                p = sps.tile([128, 128], f32, tag="tp")
                nc.tensor.transpose(p[0:C, :], delta, ident)
                nc.vector.tensor_copy(wv16[0:C, :], p[0:C, :])
                nc.vector.tensor_copy(wv16[C:C + 1, :], b1_row)

                # per tile: transpose x -> xTaug16 / xT2aug16
                for t in range(NT):
                    p = sps.tile([128, 128], f32, tag="tp")
                    nc.tensor.transpose(p[0:C, :], x_sb[:, t, :], ident)
                    sl = slice(t * 128, (t + 1) * 128)
                    nc.vector.tensor_copy(xTaug16[0:C, sl], p[0:C, :])
                    nc.vector.tensor_scalar_mul(xT2aug16[0:C, sl], p[0:C, :], 2.0)

                # -|x|^2 row: square fp16 xT, column-sum via ones matmul
                xsq = sp.tile([C, N], f32, tag="xsq")
                nc.vector.tensor_mul(xsq, xTaug16[0:C, :], xTaug16[0:C, :])
                negones = cp.tile([C, 1], f32)
                nc.vector.memset(negones, -1.0)
                sqrow = sp.tile([1, N], f32, tag="sqrow")
                for b in range(NBLK):
                    p = sps.tile([1, 512], f32, tag="sq")
                    nc.tensor.matmul(p, negones, xsq[:, b * 512:(b + 1) * 512],
                                     start=True, stop=True)
                    nc.vector.tensor_copy(sqrow[0:1, b * 512:(b + 1) * 512], p)
                # hi/lo fp16 split of sqrow (already negated); engines can't
                # address partition base 65, so stage at 0/1 and DMA in.
                hi16 = sp.tile([1, N], f16, tag="hi16")
                nc.vector.tensor_copy(hi16, sqrow)
                hi32 = sp.tile([1, N], f32, tag="hi32")
                nc.vector.tensor_copy(hi32, hi16)
                lo16 = sp.tile([1, N], f16, tag="lo16")
                nc.vector.tensor_tensor(
                    out=lo16, in0=sqrow, in1=hi32,
                    op=mybir.AluOpType.subtract)
                nc.sync.dma_start(out=xT2aug16[C:C + 1, :], in_=hi16)
                nc.sync.dma_start(out=xT2aug16[C + 1:CA, :], in_=lo16)

                # u = W1e x per point; vT -> [ch, T, pt]
                for t in range(NT):
                    sl = slice(t * 128, (t + 1) * 128)
                    if GATHER == "ap":
                        pu = sps.tile([128, 128], f32, tag="tp")
                        nc.tensor.matmul(pu, w1t16[0:C, :], xTaug16[0:C, sl],
                                         start=True, stop=True)
                        nc.vector.tensor_copy(uT_sb[:, sl], pu)
                    else:
                        pu = sps.tile([128, 128], f32, tag="tp")
                        nc.tensor.matmul(pu, xTaug16[0:C, sl], w1t16[0:C, :],
                                         start=True, stop=True)
                        nc.vector.tensor_copy(u_stage[:, t, :], pu)
                    pv = sps.tile([128, 128], f32, tag="tp")
                    nc.tensor.matmul(pv, wv16, xTaug16[:, sl],
                                     start=True, stop=True)
                    nc.vector.tensor_copy(vT_all[:, t, :], pv)

            # ---- per-tile index list + wrapped gather lists
            ilist = cp.tile([128, nt, 32], i16)
            wrap = cp.tile([128, nt, 160], i16)
            pooledT = cp.tile([C3, nt, 128], f32)   # pooled^T accum [ch, T, pt]
            gather_dep = [False]

            with tc.tile_pool(name="topk", bufs=2) as tk, \
                 tc.tile_pool(name="mlp", bufs=3) as mp, \
                 tc.tile_pool(name="h2p", bufs=2) as h2p, \
                 tc.tile_pool(name="h3p", bufs=2) as h3p, \
                 tc.tile_pool(name="ps_dist", bufs=2, space="PSUM") as psd, \
                 tc.tile_pool(name="ps_l2", bufs=2, space="PSUM") as ps2, \
                 tc.tile_pool(name="ps_l3", bufs=2, space="PSUM") as ps3:

                def topk_tile(t):
                    m_sb = tk.tile([128, NCAND], f32, tag="m_sb")
                    lidx = tk.tile([128, NCAND], u16, tag="lidx")
                    tsl = slice(t * 128, (t + 1) * 128)
                    for b in range(NBLK):
                        pd = psd.tile([128, 512], f32, tag="dist")
                        nc.tensor.matmul(
                            pd, xTaug16[:, tsl],
                            xT2aug16[:, b * 512:(b + 1) * 512],
                            start=True, stop=True)
                        nc.vector.max(out=m_sb[:, b * 8:(b + 1) * 8], in_=pd)
                        nc.vector.max_index(
                            out=lidx[:, b * 8:(b + 1) * 8],
                            in_max=m_sb[:, b * 8:(b + 1) * 8], in_values=pd)
                    # merge: top-20 of 64 -> per-row compacted gather list
                    work = tk.tile([128, NCAND], f32, tag="work")
                    nc.vector.tensor_copy(work, m_sb)
                    t8 = tk.tile([128, 8], f32, tag="t8")
                    for _ in range(2):
                        nc.vector.max(out=t8, in_=work)
                        nc.vector.match_replace(
                            out=work, in_to_replace=t8, in_values=work,
                            imm_value=NEG)
                    nc.vector.max(out=t8, in_=work)
                    qual = tk.tile([128, NCAND], f32, tag="qual")
                    v20b = bass.AP(tensor=t8.tensor,
                                   offset=t8.offset + 3 * t8.ap[-1][0],
                                   ap=[t8.ap[0], [0, NCAND]])
                    nc.vector.tensor_tensor(
                        out=qual, in0=m_sb, in1=v20b,
                        op=mybir.AluOpType.is_ge)
                    pos = tk.tile([128, NCAND], f32, tag="pos")
                    nc.vector.tensor_tensor_scan(
                        out=pos, data0=qual, data1=qual, initial=0.0,
                        op0=mybir.AluOpType.add, op1=mybir.AluOpType.bypass)
                    nc.vector.tensor_mul(pos, pos, qual)
                    nc.vector.tensor_scalar(
                        pos, pos, 1.0, scalar2=31.0,
                        op0=mybir.AluOpType.subtract, op1=mybir.AluOpType.min)
                    pidx = tk.tile([128, NCAND], i16, tag="pidx")
                    nc.vector.tensor_copy(pidx, pos)
                    gidx = tk.tile([128, NCAND], i16, tag="gidx")
                    nc.vector.tensor_tensor(
                        out=gidx, in0=lidx, in1=offs,
                        op=mybir.AluOpType.add)
                    sc = nc.gpsimd.local_scatter(
                        out_ap=ilist[:, t, :], data_ap=gidx, idxs_ap=pidx,
                        channels=128, num_elems=32, num_idxs=NCAND)
                    chain.append(sc.ins)

                def fold_group(tiles):
                    # rewrap: wrap[q, T, h*20+k] = ilist[16h+q, T, k]
                    for h in range(8):
                        nc.sync.dma_start(
                            out=wrap[0:16, tiles[0]:tiles[-1] + 1,
                                     h * 20:(h + 1) * 20],
                            in_=ilist[16 * h:16 * (h + 1),
                                      tiles[0]:tiles[-1] + 1, 0:20])
                    for g2 in range(1, 8):
                        nc.sync.dma_start(
                            out=wrap[16 * g2:16 * (g2 + 1),
                                     tiles[0]:tiles[-1] + 1, :],
                            in_=wrap[0:16, tiles[0]:tiles[-1] + 1, :])

                def gather_tile(t):
                    if GATHER == "ap":
                        ug = mp.tile([128, PAIRS], f32, tag="ug")
                        gi = nc.gpsimd.ap_gather(
                            out_ap=ug, in_ap=uT_sb, idxs_ap=wrap[:, t, :],
                            channels=128, num_elems=N, d=1, num_idxs=PAIRS)
                    else:
                        ug = mp.tile([128, PAIRS], f16, tag="ug")
                        gi = nc.gpsimd.dma_gather(
                            out_ap=ug.rearrange("p (a q) -> p a q", a=1),
                            in_ap=u_stage.rearrange("p t c -> p (t c)"),
                            idxs_ap=wrap[:, t, :],
                            num_idxs=PAIRS,
                            num_idxs_reg=PAIRS,
                            elem_size=C1,
                            transpose=True,
                            sbuf_tokens_per_rank=128,
                            sbuf_free_dim_per_rank=256,
                            sbuf_free_dim_pad_per_rank=0,
                            sbuf_byte_offset=0,
                            single_packet=SINGLE_PACKET,
                        )
                    chain.append(gi.ins)
                    return ug

                def mlp_pool_tile(t, ug):
                    # L1: h1 = relu(u_j + v_i), broadcast v over k.
                    # Gather pair order is (h, k, q): pair = h*320 + k*16 + q,
                    # point i = 16h + q -> v view strides (16, 0, 1) elems.
                    h1p = mp.tile([128, PAIRS], f16, tag="h1p")
                    vt = vT_all[:, t, :]
                    s = vt.ap[-1][0]
                    if VEXP_DMA:
                        # materialize the k-broadcast by DMA so the DVE add
                        # runs on two contiguous fp16 operands (2x mode)
                        vexp = mp.tile([128, PAIRS], f16, tag="vexp")
                        sv = vexp.ap[-1][0]
                        nc.sync.dma_start(
                            out=bass.AP(tensor=vexp.tensor, offset=vexp.offset,
                                        ap=[vexp.ap[0], [320 * sv, 8],
                                            [16 * sv, K], [sv, 16]]),
                            in_=bass.AP(tensor=vt.tensor, offset=vt.offset,
                                        ap=[vt.ap[0], [16 * s, 8], [0, K],
                                            [s, 16]]))
                        nc.vector.tensor_tensor(
                            out=h1p, in0=ug, in1=vexp,
                            op=mybir.AluOpType.add)
                    else:
                        vb = bass.AP(tensor=vt.tensor, offset=vt.offset,
                                     ap=[vt.ap[0], [16 * s, 8], [0, K],
                                         [s, 16]])
                        sh = h1p.ap[-1][0]
                        h1pv = bass.AP(tensor=h1p.tensor, offset=h1p.offset,
                                       ap=[h1p.ap[0], [320 * sh, 8],
                                           [16 * sh, K], [sh, 16]])
                        su = ug.ap[-1][0]
                        ugv = bass.AP(tensor=ug.tensor, offset=ug.offset,
                                      ap=[ug.ap[0], [320 * su, 8],
                                          [16 * su, K], [su, 16]])
                        nc.vector.tensor_tensor(
                            out=h1pv, in0=ugv, in1=vb, op=mybir.AluOpType.add)
                    h1 = mp.tile([128, PAIRS], f16, tag="h1")
                    nc.scalar.activation(h1, h1p,
                                         mybir.ActivationFunctionType.Relu)
                    # L2
                    h2 = h2p.tile([C2, PAIRS], f16, tag="h2")
                    for j in range(5):
                        sl = slice(j * 512, (j + 1) * 512)
                        p2 = ps2.tile([C2, 512], f32, tag="l2")
                        nc.tensor.matmul(p2, w2t16, h1[:, sl],
                                         start=True, stop=True)
                        nc.scalar.activation(
                            h2[:, sl], p2, mybir.ActivationFunctionType.Relu,
                            bias=b2_col)
                    # L3
                    h3 = h3p.tile([C3, PAIRS], f16, tag="h3")
                    for j in range(5):
                        sl = slice(j * 512, (j + 1) * 512)
                        p3 = ps3.tile([C3, 512], f32, tag="l3")
                        nc.tensor.matmul(p3, w3t16, h2[:, sl],
                                         start=True, stop=True)
                        nc.scalar.activation(
                            h3[:, sl], p3, mybir.ActivationFunctionType.Relu,
                            bias=b3_col)
                    # pool over k (tree on fp16 strided k-views)
                    def kview(src, nk, k0, kn):
                        s = src.ap[-1][0]
                        return bass.AP(
                            tensor=src.tensor,
                            offset=src.offset + k0 * 16 * s,
                            ap=[src.ap[0], [s * 16 * nk, 8], [s * 16, kn],
                                [s, 16]])
                    m10 = h3p.tile([C3, 1280], f16, tag="m10")
                    nc.vector.tensor_max(
                        m10, kview(h3, K, 0, 10), kview(h3, K, 10, 10))
                    m5 = h3p.tile([C3, 640], f16, tag="m5")
                    nc.vector.tensor_max(
                        m5, kview(m10, 10, 0, 5), kview(m10, 10, 5, 5))
                    m2 = h3p.tile([C3, 256], f16, tag="m2")
                    nc.vector.tensor_max(
                        m2, kview(m5, 5, 0, 2), kview(m5, 5, 2, 2))
                    m1 = h3p.tile([C3, 128], f16, tag="m1")
                    nc.vector.tensor_max(
                        m1, kview(m2, 2, 0, 1), kview(m2, 2, 1, 1))
                    nc.vector.tensor_max(pooledT[:, t, :], m1,
                                         kview(m5, 5, 4, 1))

                n_groups = (nt + GROUP - 1) // GROUP
                for g in range(n_groups):
                    tiles = list(range(g * GROUP, min((g + 1) * GROUP, nt)))
                    for t in tiles:
                        topk_tile(t)
                    fold_group(tiles)
                    ugs = {t: gather_tile(t) for t in tiles}
                    for t in tiles:
                        mlp_pool_tile(t, ugs[t])

            # tail: transpose pooled^T -> [pt, ch], accumulate, one DMA out
            with tc.tile_pool(name="outp", bufs=2) as op_, \
                 tc.tile_pool(name="ps_ot", bufs=2, space="PSUM") as pso:
                osb = cp.tile([128, nt, C3], f32)
                for t in range(nt):
                    po = pso.tile([128, C3], f32, tag="ot")
                    nc.tensor.transpose(po, pooledT[:, t, :],
                                        ident[0:C3, 0:C3])
                    nc.vector.tensor_copy(osb[:, t, :], po)
                nc.sync.dma_start(out=bass.AP(
                    tensor=out.ap().tensor, offset=0,
                    ap=[[C3, 128], [128 * C3, nt], [1, C3]]), in_=osb)

        if os.environ.get("KM_CHAIN", "1") == "1":
            from concourse.tile_rust import add_dep_helper
            for a, b_ in zip(chain, chain[1:]):
                add_dep_helper(b_, a, sync=False, reason="gpsimd library batching")

    return nc


_nc_cache = None
LAST_EXEC_NS = None
LAST_TRACE = None


def kernel(points, W1, b1, W2, b2, W3, b3):
    global _nc_cache
    if _nc_cache is None:
        _nc_cache = build_nc()
        _nc_cache.finalize()
    nc = _nc_cache
    common = {
        "W1": np.ascontiguousarray(W1, dtype=np.float32),
        "b1": np.ascontiguousarray(b1, dtype=np.float32),
        "W2": np.ascontiguousarray(W2, dtype=np.float32),
        "b2": np.ascontiguousarray(b2, dtype=np.float32),
        "W3": np.ascontiguousarray(W3, dtype=np.float32),
        "b3": np.ascontiguousarray(b3, dtype=np.float32),
    }
    in_maps = [
        dict(common, points=np.ascontiguousarray(points[b], dtype=np.float32))
        for b in range(B)
    ]
    trace = os.environ.get("BASS_TRACE", "0") == "1"
    res = run_bass_kernel_spmd(nc, in_maps, list(range(B)), trace=trace)
    global LAST_EXEC_NS, LAST_TRACE
    LAST_EXEC_NS = res.exec_time_ns
    LAST_TRACE = res.instructions_and_trace
    return np.stack([res.results[b]["out"] for b in range(B)], axis=0)


if __name__ == "__main__":
    pts = np.random.randn(B, N, C).astype(np.float32)
    W1_ = (np.random.randn(C1, 2 * C) * 0.05).astype(np.float32)
    W2_ = (np.random.randn(C2, C1) * 0.05).astype(np.float32)
    W3_ = (np.random.randn(C3, C2) * 0.05).astype(np.float32)
    z1, z2, z3 = (np.zeros(C1, np.float32), np.zeros(C2, np.float32),
                  np.zeros(C3, np.float32))
    o = kernel(pts, W1_, z1, W2_, z2, W3_, z3)
    print(o.shape, o.dtype)
